# revision 10
# baseline (speedup 1.0000x reference)
"""DTCWT inverse (qshift, single level) as a Bass/Tile kernel for TRN2.

Factorization (column filter first):
    out = (C0·Yl + C1·lh)·C0^T + (C0·hl + C1·hh)·C1^T
with C0/C1 the 256x128 banded synthesis (colifilt) matrices; the rowifilt
matrices are identical, so the same filter bank serves both stages.

Quadrant columns are kept BLOCKED ([even spatial cols | odd]) instead of
interleaved; the column permutation is absorbed into the row order of the
stage-2 statics (and Yl's columns are pre-permuted on the host). With
blocked columns the c2q butterfly needs no data duplication:
    per band pair:  Ra = [w1r | w1i],  Rb = [w2r | w2i]   (64 x 128 raw)
    even quad rows: Ra + Rb          odd rows: swapsig(Ra - Rb)
where swapsig(X) = [X[:,64:] | -X[:,:64]] is two contiguous half-copies.
Band data crosses HBM exactly once (except the lh pair, which is sent
pre-stacked 2x so its even/odd halves are partition-stacked for the PE).

Per slice, two PE stages of 4 matmuls each (256 moving rows, 1 cyc/row):
    stage 1: z^T[c',i] psum [128, 512]=[z1^T|z2^T]; stationaries Yl, A1
             (lh even;odd), A2 (hl;hh even), A3 (hl;hh odd) vs stacked
             row-parity statics
    stage 2: out[i-tile] = zs-half^T @ C*T_c (c'-row-permuted statics)

All bf16 with f32 PSUM accumulation; rel err ~6e-3 vs f32 ref (gate 2e-2).

Inputs ship in ONE packed dram tensor IN[p, slice, 5, 128] (Yl | U1 | V1 |
U2 | V2) = one dma_start per group (~650ns issue each), >=1.25KB
contiguous per-partition lines. Small starter/final groups shorten
pipeline fill/drain. Engine budget per slice: PE 8 matmuls, DVE 3 adds +
out-copy, ACT z-copy + 2 half-copies (A3), sync all DMA issues.
"""
import numpy as np
import ml_dtypes

import concourse.bacc as bacc
import concourse.tile as tile
from concourse import mybir

F32 = mybir.dt.float32
BF16 = mybir.dt.bfloat16
NPBF16 = ml_dtypes.bfloat16

PERM = np.concatenate([np.arange(0, 128, 2), np.arange(1, 128, 2)])

# ---------------- host-side static matrix construction ----------------

_H0A = np.array([0.0351638365171441, 0.0, -0.0883294244510729,
                 0.233890320607236, 0.760272369066126, 0.587518297723561,
                 0.0, -0.114301837144249, 0.0, 0.0], dtype=np.float64)
_H0B = _H0A[::-1].copy()
_ALT = (-1.0) ** np.arange(10)
_H1A = _H0B * _ALT
_H1B = _H1A[::-1].copy()
G0A, G0B, G1A, G1B = _H0B, _H0A, _H1B, _H1A


def _reflect(x, minx, maxx):
    x = np.asarray(x, dtype=np.float64)
    rng = maxx - minx
    rng2 = 2.0 * rng
    mod = np.fmod(x - minx, rng2)
    normed = np.where(mod < 0, mod + rng2, mod)
    return (np.where(normed >= rng, rng2 - normed, normed) + minx).astype(np.int64)


def _colifilt_matrix(ha, hb, r=128):
    """C (2r x r) with colifilt(X) = C @ X."""
    m = ha.shape[0]
    m2 = m // 2
    xe = _reflect(np.arange(-m2, r + m2), -0.5, r - 0.5)
    t = np.arange(2, r + m - 1, 2)
    if float(np.sum(ha * hb)) > 0:
        ta, tb = t, t - 1
    else:
        ta, tb = t - 1, t
    r2 = r // 2
    hao, hae = ha[0::2], ha[1::2]
    hbo, hbe = hb[0::2], hb[1::2]

    def vconv_mat(sel_idx, h):
        hf = h[::-1]
        M = np.zeros((r2, r), dtype=np.float64)
        for i in range(r2):
            for k in range(m2):
                M[i, sel_idx[i + k]] += hf[k]
        return M

    C = np.zeros((2 * r, r), dtype=np.float64)
    C[0::4] = vconv_mat(xe[tb], hao)
    C[1::4] = vconv_mat(xe[ta], hbo)
    C[2::4] = vconv_mat(xe[tb], hae)
    C[3::4] = vconv_mat(xe[ta], hbe)
    return C


def build_statics():
    """CT [128, 1536] bf16 = [C0T | C1s_EO | Cs_E2 | Cs_O2 | C0T_c | C1T_c]."""
    C0 = _colifilt_matrix(G0B, G0A)
    C1 = _colifilt_matrix(G1B, G1A)
    s = 1.0 / np.sqrt(2.0)
    C0T = C0.T
    C1T = C1.T
    C0sT = (s * C0).T
    C1sT = (s * C1).T
    C1s_EO = np.concatenate([C1sT[0::2], C1sT[1::2]], axis=0)
    Cs_E2 = np.concatenate([C0sT[0::2], C1sT[0::2]], axis=0)
    Cs_O2 = np.concatenate([C0sT[1::2], C1sT[1::2]], axis=0)
    CT = np.concatenate(
        [C0T, C1s_EO, Cs_E2, Cs_O2, C0T[PERM], C1T[PERM]], axis=1)
    return np.ascontiguousarray(CT.astype(np.float32).astype(NPBF16))


# ---------------- device kernel ----------------


def build_kernel(n_ch=64, G=8, n_cores=8):
    nc = bacc.Bacc("TRN2", target_bir_lowering=False, debug=False,
                   num_devices=n_cores)
    IN = nc.dram_tensor("IN", [128, n_ch, 5, 128], BF16, kind="ExternalInput").ap()
    CTD = nc.dram_tensor("CT", [128, 1536], BF16, kind="ExternalInput").ap()
    OUT = nc.dram_tensor("Y", [n_ch, 256, 256], BF16, kind="ExternalOutput").ap()

    # group schedule: tiny starter groups so the first matmuls fire early,
    # small final groups so the output-store drain tail is short
    groups = [(0, 2), (2, 2), (4, 4)]
    g0 = 8
    while g0 < n_ch - G:
        groups.append((g0, G))
        g0 += G
    groups += [(g0, 4), (g0 + 4, 2), (g0 + 6, 2)]
    assert sum(gl for _, gl in groups) == n_ch
    assert all(gl in (2, 4, 8) for _, gl in groups)

    with tile.TileContext(nc) as tc:
        with (
            tc.tile_pool(name="const", bufs=1) as const,
            tc.tile_pool(name="inp", bufs=3) as inp,
            tc.tile_pool(name="quad", bufs=3) as quad,
            tc.tile_pool(name="zt", bufs=4) as ztp,
            tc.tile_pool(name="yout", bufs=3) as yp,
            tc.tile_pool(name="psz", bufs=3, space="PSUM") as pp,
            tc.tile_pool(name="psy", bufs=3, space="PSUM") as ppy,
        ):
            ct = const.tile([128, 1536], BF16)
            nc.sync.dma_start(ct[:], CTD[:])
            C0T = ct[:, 0:256]
            C1s_EO = ct[:, 256:512]
            Cs_E2 = ct[:, 512:768]
            Cs_O2 = ct[:, 768:1024]
            C0Tc = ct[:, 1024:1280]
            C1Tc = ct[:, 1280:1536]

            def load_group(g0, Gl):
                tin = inp.tile([128, Gl * 5 * 128], BF16, tag=f"in{Gl}")
                nc.sync.dma_start(
                    tin.rearrange("p (g k c) -> p g k c", g=Gl, k=5),
                    IN[:, g0:g0 + Gl],
                )
                return tin, Gl

            def prep_group(state):
                tin, Gl = state
                inv = tin.rearrange("p (g k c) -> p g k c", g=Gl, k=5)
                # A-tiles: [A1 | A2 | A3] per slice, [128, 3*128] each slice
                AT = quad.tile([128, 3 * Gl * 128], BF16, tag=f"at{Gl}")
                av = AT.rearrange("p (g q c) -> p g q c", g=Gl, q=3)
                DT = quad.tile([128, Gl * 128], BF16, tag=f"dt{Gl}")
                dv = DT.rearrange("p (g h c2) -> p g h c2", g=Gl, h=2)
                nc.vector.tensor_add(av[:, :, 0], inv[:, :, 1], inv[:, :, 2])
                nc.vector.tensor_add(av[:, :, 1], inv[:, :, 3], inv[:, :, 4])
                nc.vector.tensor_sub(
                    DT.rearrange("p (g c) -> p g c", g=Gl),
                    inv[:, :, 3], inv[:, :, 4])
                a3v = av[:, :, 2].rearrange("p g (h c2) -> p g h c2", h=2)
                nc.scalar.copy(a3v[:, :, 0], dv[:, :, 1])
                nc.scalar.mul(a3v[:, :, 1], dv[:, :, 0], -1.0)
                return tin, AT, Gl

            def process_group(g0, state):
                tin, AT, Gl = state
                inv = tin.rearrange("p (g k c) -> p g k c", g=Gl, k=5)
                av = AT.rearrange("p (g q c) -> p g q c", g=Gl, q=3)
                half = min(Gl, 4)
                for h0 in range(0, Gl, half):
                    YB = yp.tile([128, half * 512], BF16, tag=f"yb{half}")
                    for j in range(half):
                        ci = h0 + j
                        zp = pp.tile([128, 512], F32, tag="zp")
                        nc.tensor.matmul(zp[:, 0:256], inv[:, ci, 0], C0T,
                                         start=True, stop=False, skip_group_check=True)
                        nc.tensor.matmul(zp[:, 0:256], av[:, ci, 0], C1s_EO,
                                         start=False, stop=False, skip_group_check=True)
                        nc.tensor.matmul(zp[:, 256:512], av[:, ci, 1], Cs_E2,
                                         start=False, stop=False, skip_group_check=True)
                        nc.tensor.matmul(zp[:, 256:512], av[:, ci, 2], Cs_O2,
                                         start=False, stop=True, skip_group_check=True)

                        zs = ztp.tile([128, 512], BF16, tag="zs")
                        nc.scalar.copy(zs[:], zp[:])

                        op = ppy.tile([128, 512], F32, tag="op")
                        nc.tensor.matmul(op[:, 0:256], zs[:, 0:128], C0Tc,
                                         start=True, stop=False, skip_group_check=True)
                        nc.tensor.matmul(op[:, 0:256], zs[:, 256:384], C1Tc,
                                         start=False, stop=False, skip_group_check=True)
                        nc.tensor.matmul(op[:, 256:512], zs[:, 128:256], C0Tc,
                                         start=False, stop=False, skip_group_check=True)
                        nc.tensor.matmul(op[:, 256:512], zs[:, 384:512], C1Tc,
                                         start=False, stop=True, skip_group_check=True)

                        nc.vector.tensor_copy(
                            YB[:, j * 512:(j + 1) * 512], op[:])

                    nc.sync.dma_start(
                        OUT[g0 + h0:g0 + h0 + half].rearrange(
                            "g (it p) w -> p g it w", it=2),
                        YB.rearrange("p (g it w) -> p g it w", g=half, it=2),
                    )

            # software pipeline: load + prep g+1 before computing g
            state = prep_group(load_group(*groups[0]))
            for idx, (g0, Gl) in enumerate(groups):
                if idx + 1 < len(groups):
                    nxt = prep_group(load_group(*groups[idx + 1]))
                    process_group(g0, state)
                    state = nxt
                else:
                    process_group(g0, state)

    nc.compile()
    return nc


# ---------------- host wrapper: shard, run on 8 cores, gather ----------------

_CACHED = {}


def _get_compiled():
    if "nc" not in _CACHED:
        _CACHED["nc"] = build_kernel(n_ch=64, G=8, n_cores=8)
        _CACHED["ct"] = build_statics()
    return _CACHED["nc"], _CACHED["ct"]


def _make_in_maps(Yl, Yhr, Yhi, CT):
    """Per-core input packing (pure layout: transpose/block/sign).
    IN[b] = [128(p), C, 5, 128]: k=0 Yl (cols blocked), k=1 U1, k=2 V1
    (lh pair, even/odd-stacked), k=3 U2=[Ra_hl;Ra_hh], k=4 V2=[Rb_hl;Rb_hh]."""
    B = Yl.shape[0]
    IN = np.empty((B, 128, 64, 5, 128), dtype=np.float32)
    IN[:, :, :, 0, :] = Yl[:, :, :, PERM].transpose(0, 2, 1, 3)

    def Rab(b):  # [B, 64(h), C, 128] = [wr | wi] blocked
        return np.concatenate(
            [Yhr[:, :, b], Yhi[:, :, b]], axis=-1).transpose(0, 2, 1, 3)

    def swapsig(X):
        return np.concatenate([X[..., 64:128], -X[..., 0:64]], axis=-1)

    Ra_lh, Rb_lh = Rab(0), Rab(5)
    IN[:, 0:64, :, 1, :] = Ra_lh
    IN[:, 64:128, :, 1, :] = swapsig(Ra_lh)
    IN[:, 0:64, :, 2, :] = Rb_lh
    IN[:, 64:128, :, 2, :] = -swapsig(Rb_lh)
    IN[:, 0:64, :, 3, :] = Rab(2)     # Ra_hl
    IN[:, 64:128, :, 3, :] = Rab(1)   # Ra_hh
    IN[:, 0:64, :, 4, :] = Rab(3)     # Rb_hl
    IN[:, 64:128, :, 4, :] = Rab(4)   # Rb_hh
    IN = IN.astype(NPBF16)
    return [{"IN": np.ascontiguousarray(IN[b]), "CT": CT} for b in range(B)]


def kernel(Yl, Yhr, Yhi):
    """Inverse DTCWT (qshift) level. Yl (8,64,128,128) f32,
    Yhr/Yhi (8,64,6,64,64) f32 -> (8,64,256,256) f32.
    Data-parallel over the batch dim: one batch element per NeuronCore."""
    from concourse.bass_utils import run_bass_kernel_spmd

    Yl = np.asarray(Yl, dtype=np.float32)
    Yhr = np.asarray(Yhr, dtype=np.float32)
    Yhi = np.asarray(Yhi, dtype=np.float32)
    B = Yl.shape[0]
    assert B == 8, f"expected batch 8, got {B}"

    nc, CT = _get_compiled()
    in_maps = _make_in_maps(Yl, Yhr, Yhi, CT)
    res = run_bass_kernel_spmd(nc, in_maps, core_ids=list(range(B)))
    out = np.stack([np.asarray(res.results[b]["Y"]) for b in range(B)])
    return out.astype(np.float32)


# revision 14
# speedup vs baseline: 1.0698x; 1.0698x over previous
"""DTCWT inverse (qshift, single level) as a Bass/Tile kernel for TRN2.

Factorization (column filter first):
    out = (C0·Yl + C1·lh)·C0^T + (C0·hl + C1·hh)·C1^T
with C0/C1 the 256x128 banded synthesis (colifilt) matrices; the rowifilt
matrices are identical, so the same filter bank serves both stages.

Quadrant columns are kept BLOCKED ([even spatial cols | odd]) instead of
interleaved; the column permutation is absorbed into the row order of the
stage-2 statics (and Yl's columns are pre-permuted on the host). With
blocked columns the c2q butterfly needs no data duplication:
    per band pair:  Ra = [w1r | w1i],  Rb = [w2r | w2i]   (64 x 128 raw)
    even quad rows: Ra + Rb          odd rows: swapsig(Ra - Rb)
where swapsig(X) = [X[:,64:] | -X[:,:64]] is two contiguous half-copies.
Band data crosses HBM exactly once (except the lh pair, which is sent
pre-stacked 2x so its even/odd halves are partition-stacked for the PE).

Per slice, two PE stages of 4 matmuls each (256 moving rows, 1 cyc/row):
    stage 1: z^T[c',i] psum [128, 512]=[z1^T|z2^T]; stationaries Yl, A1
             (lh even;odd), A2 (hl;hh even), A3 (hl;hh odd) vs stacked
             row-parity statics
    stage 2: out[i-tile] = zs-half^T @ C*T_c (c'-row-permuted statics)

All bf16 with f32 PSUM accumulation; rel err ~6e-3 vs f32 ref (gate 2e-2).

Inputs ship in ONE packed dram tensor IN[p, slice, 5, 128] (Yl | U1 | V1 |
U2 | V2) = one dma_start per group (~650ns issue each), >=1.25KB
contiguous per-partition lines. Small starter/final groups shorten
pipeline fill/drain. Engine budget per slice: PE 8 matmuls, DVE 3 adds +
out-copy, ACT z-copy + 2 half-copies (A3), sync all DMA issues.
"""
import numpy as np
import ml_dtypes

import concourse.bacc as bacc
import concourse.tile as tile
from concourse import mybir

F32 = mybir.dt.float32
BF16 = mybir.dt.bfloat16
NPBF16 = ml_dtypes.bfloat16

PERM = np.concatenate([np.arange(0, 128, 2), np.arange(1, 128, 2)])

# ---------------- host-side static matrix construction ----------------

_H0A = np.array([0.0351638365171441, 0.0, -0.0883294244510729,
                 0.233890320607236, 0.760272369066126, 0.587518297723561,
                 0.0, -0.114301837144249, 0.0, 0.0], dtype=np.float64)
_H0B = _H0A[::-1].copy()
_ALT = (-1.0) ** np.arange(10)
_H1A = _H0B * _ALT
_H1B = _H1A[::-1].copy()
G0A, G0B, G1A, G1B = _H0B, _H0A, _H1B, _H1A


def _reflect(x, minx, maxx):
    x = np.asarray(x, dtype=np.float64)
    rng = maxx - minx
    rng2 = 2.0 * rng
    mod = np.fmod(x - minx, rng2)
    normed = np.where(mod < 0, mod + rng2, mod)
    return (np.where(normed >= rng, rng2 - normed, normed) + minx).astype(np.int64)


def _colifilt_matrix(ha, hb, r=128):
    """C (2r x r) with colifilt(X) = C @ X."""
    m = ha.shape[0]
    m2 = m // 2
    xe = _reflect(np.arange(-m2, r + m2), -0.5, r - 0.5)
    t = np.arange(2, r + m - 1, 2)
    if float(np.sum(ha * hb)) > 0:
        ta, tb = t, t - 1
    else:
        ta, tb = t - 1, t
    r2 = r // 2
    hao, hae = ha[0::2], ha[1::2]
    hbo, hbe = hb[0::2], hb[1::2]

    def vconv_mat(sel_idx, h):
        hf = h[::-1]
        M = np.zeros((r2, r), dtype=np.float64)
        for i in range(r2):
            for k in range(m2):
                M[i, sel_idx[i + k]] += hf[k]
        return M

    C = np.zeros((2 * r, r), dtype=np.float64)
    C[0::4] = vconv_mat(xe[tb], hao)
    C[1::4] = vconv_mat(xe[ta], hbo)
    C[2::4] = vconv_mat(xe[tb], hae)
    C[3::4] = vconv_mat(xe[ta], hbe)
    return C


def build_statics():
    """CT [128, 1536] bf16 = [C0T | C1s_EO | Cs_E2 | Cs_O2 | C0T_c | C1T_c]."""
    C0 = _colifilt_matrix(G0B, G0A)
    C1 = _colifilt_matrix(G1B, G1A)
    s = 1.0 / np.sqrt(2.0)
    C0T = C0.T
    C1T = C1.T
    C0sT = (s * C0).T
    C1sT = (s * C1).T
    C1s_EO = np.concatenate([C1sT[0::2], C1sT[1::2]], axis=0)
    Cs_E2 = np.concatenate([C0sT[0::2], C1sT[0::2]], axis=0)
    Cs_O2 = np.concatenate([C0sT[1::2], C1sT[1::2]], axis=0)
    CT = np.concatenate(
        [C0T, C1s_EO, Cs_E2, Cs_O2, C0T[PERM], C1T[PERM]], axis=1)
    return np.ascontiguousarray(CT.astype(np.float32).astype(NPBF16))


# ---------------- device kernel ----------------


def build_kernel(n_ch=64, G=8, n_cores=8):
    nc = bacc.Bacc("TRN2", target_bir_lowering=False, debug=False,
                   num_devices=n_cores)
    IN = nc.dram_tensor("IN", [128, 5, n_ch, 128], BF16, kind="ExternalInput").ap()
    CTD = nc.dram_tensor("CT", [128, 1536], BF16, kind="ExternalInput").ap()
    OUT = nc.dram_tensor("Y", [n_ch, 256, 256], BF16, kind="ExternalOutput").ap()

    # group schedule: tiny starter groups so the first matmuls fire early,
    # small final groups so the output-store drain tail is short
    groups = [(0, 2), (2, 2), (4, 4)]
    g0 = 8
    while g0 < n_ch - G:
        groups.append((g0, G))
        g0 += G
    groups += [(g0, 4), (g0 + 4, 2), (g0 + 6, 2)]
    assert sum(gl for _, gl in groups) == n_ch
    assert all(gl in (2, 4, 8) for _, gl in groups)

    with tile.TileContext(nc) as tc:
        with (
            tc.tile_pool(name="const", bufs=1) as const,
            tc.tile_pool(name="inp", bufs=3) as inp,
            tc.tile_pool(name="quad", bufs=3) as quad,
            tc.tile_pool(name="zt", bufs=4) as ztp,
            tc.tile_pool(name="yout", bufs=3) as yp,
            tc.tile_pool(name="psz", bufs=3, space="PSUM") as pp,
            tc.tile_pool(name="psy", bufs=3, space="PSUM") as ppy,
        ):
            ct = const.tile([128, 1536], BF16)
            nc.sync.dma_start(ct[:], CTD[:])
            C0T = ct[:, 0:256]
            C1s_EO = ct[:, 256:512]
            Cs_E2 = ct[:, 512:768]
            Cs_O2 = ct[:, 768:1024]
            C0Tc = ct[:, 1024:1280]
            C1Tc = ct[:, 1280:1536]

            def load_group(g0, Gl):
                tin = inp.tile([128, 5 * Gl * 128], BF16, tag=f"in{Gl}")
                nc.sync.dma_start(
                    tin.rearrange("p (k g c) -> p k g c", k=5, g=Gl),
                    IN[:, :, g0:g0 + Gl],
                )
                return tin, Gl

            def prep_group(state):
                tin, Gl = state
                S = Gl * 128
                # A-tiles: [A1 slices | A2 slices | A3 slices], contiguous
                AT = quad.tile([128, 3 * S], BF16, tag=f"at{Gl}")
                DT = quad.tile([128, S], BF16, tag=f"dt{Gl}")
                nc.vector.tensor_add(AT[:, 0:S], tin[:, S:2 * S], tin[:, 2 * S:3 * S])
                nc.vector.tensor_add(AT[:, S:2 * S], tin[:, 3 * S:4 * S],
                                     tin[:, 4 * S:5 * S])
                nc.vector.tensor_sub(DT[:], tin[:, 3 * S:4 * S], tin[:, 4 * S:5 * S])
                dv = DT.rearrange("p (g h c2) -> p g h c2", g=Gl, h=2)
                a3v = AT[:, 2 * S:3 * S].rearrange(
                    "p (g h c2) -> p g h c2", g=Gl, h=2)
                nc.scalar.copy(a3v[:, :, 0], dv[:, :, 1])
                nc.scalar.mul(a3v[:, :, 1], dv[:, :, 0], -1.0)
                return tin, AT, Gl

            def process_group(g0, state):
                tin, AT, Gl = state
                S = Gl * 128
                inv = tin.rearrange("p (k g c) -> p k g c", k=5, g=Gl)
                av = AT.rearrange("p (q g c) -> p q g c", q=3, g=Gl)
                half = min(Gl, 4)
                for h0 in range(0, Gl, half):
                    YB = yp.tile([128, half * 512], BF16, tag=f"yb{half}")
                    for j in range(half):
                        ci = h0 + j
                        zp = pp.tile([128, 512], F32, tag="zp")
                        nc.tensor.matmul(zp[:, 0:256], inv[:, 0, ci], C0T,
                                         start=True, stop=False, skip_group_check=True)
                        nc.tensor.matmul(zp[:, 0:256], av[:, 0, ci], C1s_EO,
                                         start=False, stop=False, skip_group_check=True)
                        nc.tensor.matmul(zp[:, 256:512], av[:, 1, ci], Cs_E2,
                                         start=False, stop=False, skip_group_check=True)
                        nc.tensor.matmul(zp[:, 256:512], av[:, 2, ci], Cs_O2,
                                         start=False, stop=True, skip_group_check=True)

                        zs = ztp.tile([128, 512], BF16, tag="zs")
                        nc.scalar.copy(zs[:], zp[:])

                        op = ppy.tile([128, 512], F32, tag="op")
                        nc.tensor.matmul(op[:, 0:256], zs[:, 0:128], C0Tc,
                                         start=True, stop=False, skip_group_check=True)
                        nc.tensor.matmul(op[:, 0:256], zs[:, 256:384], C1Tc,
                                         start=False, stop=False, skip_group_check=True)
                        nc.tensor.matmul(op[:, 256:512], zs[:, 128:256], C0Tc,
                                         start=False, stop=False, skip_group_check=True)
                        nc.tensor.matmul(op[:, 256:512], zs[:, 384:512], C1Tc,
                                         start=False, stop=True, skip_group_check=True)

                        nc.vector.tensor_copy(
                            YB[:, j * 512:(j + 1) * 512], op[:])

                    nc.sync.dma_start(
                        OUT[g0 + h0:g0 + h0 + half].rearrange(
                            "g (it p) w -> p g it w", it=2),
                        YB.rearrange("p (g it w) -> p g it w", g=half, it=2),
                    )

            # software pipeline: load + prep g+1 before computing g
            state = prep_group(load_group(*groups[0]))
            for idx, (g0, Gl) in enumerate(groups):
                if idx + 1 < len(groups):
                    nxt = prep_group(load_group(*groups[idx + 1]))
                    process_group(g0, state)
                    state = nxt
                else:
                    process_group(g0, state)

    nc.compile()
    return nc


# ---------------- host wrapper: shard, run on 8 cores, gather ----------------

_CACHED = {}


def _get_compiled():
    if "nc" not in _CACHED:
        _CACHED["nc"] = build_kernel(n_ch=64, G=8, n_cores=8)
        _CACHED["ct"] = build_statics()
    return _CACHED["nc"], _CACHED["ct"]


def _make_in_maps(Yl, Yhr, Yhi, CT):
    """Per-core input packing (pure layout: transpose/block/sign).
    IN[b] = [128(p), C, 5, 128]: k=0 Yl (cols blocked), k=1 U1, k=2 V1
    (lh pair, even/odd-stacked), k=3 U2=[Ra_hl;Ra_hh], k=4 V2=[Rb_hl;Rb_hh]."""
    B = Yl.shape[0]
    IN = np.empty((B, 128, 5, 64, 128), dtype=np.float32)
    IN[:, :, 0] = Yl[:, :, :, PERM].transpose(0, 2, 1, 3)

    def Rab(b):  # [B, 64(h), C, 128] = [wr | wi] blocked
        return np.concatenate(
            [Yhr[:, :, b], Yhi[:, :, b]], axis=-1).transpose(0, 2, 1, 3)

    def swapsig(X):
        return np.concatenate([X[..., 64:128], -X[..., 0:64]], axis=-1)

    Ra_lh, Rb_lh = Rab(0), Rab(5)
    IN[:, 0:64, 1] = Ra_lh
    IN[:, 64:128, 1] = swapsig(Ra_lh)
    IN[:, 0:64, 2] = Rb_lh
    IN[:, 64:128, 2] = -swapsig(Rb_lh)
    IN[:, 0:64, 3] = Rab(2)     # Ra_hl
    IN[:, 64:128, 3] = Rab(1)   # Ra_hh
    IN[:, 0:64, 4] = Rab(3)     # Rb_hl
    IN[:, 64:128, 4] = Rab(4)   # Rb_hh
    IN = IN.astype(NPBF16)
    return [{"IN": np.ascontiguousarray(IN[b]), "CT": CT} for b in range(B)]


def kernel(Yl, Yhr, Yhi):
    """Inverse DTCWT (qshift) level. Yl (8,64,128,128) f32,
    Yhr/Yhi (8,64,6,64,64) f32 -> (8,64,256,256) f32.
    Data-parallel over the batch dim: one batch element per NeuronCore."""
    from concourse.bass_utils import run_bass_kernel_spmd

    Yl = np.asarray(Yl, dtype=np.float32)
    Yhr = np.asarray(Yhr, dtype=np.float32)
    Yhi = np.asarray(Yhi, dtype=np.float32)
    B = Yl.shape[0]
    assert B == 8, f"expected batch 8, got {B}"

    nc, CT = _get_compiled()
    in_maps = _make_in_maps(Yl, Yhr, Yhi, CT)
    res = run_bass_kernel_spmd(nc, in_maps, core_ids=list(range(B)))
    out = np.stack([np.asarray(res.results[b]["Y"]) for b in range(B)])
    return out.astype(np.float32)


# revision 16
# speedup vs baseline: 1.1562x; 1.0808x over previous
"""DTCWT inverse (qshift, single level) as a Bass/Tile kernel for TRN2.

Factorization (column filter first):
    out = (C0·Yl + C1·lh)·C0^T + (C0·hl + C1·hh)·C1^T
with C0/C1 the 256x128 banded synthesis (colifilt) matrices; the rowifilt
matrices are identical, so the same filter bank serves both stages.

Quadrant columns are kept BLOCKED ([even spatial cols | odd]) instead of
interleaved; the column permutation is absorbed into the row order of the
stage-2 statics (and Yl's columns are pre-permuted on the host). With
blocked columns the c2q butterfly needs no data duplication:
    per band pair:  Ra = [w1r | w1i],  Rb = [w2r | w2i]   (64 x 128 raw)
    even quad rows: Ra + Rb          odd rows: swapsig(Ra - Rb)
where swapsig(X) = [X[:,64:] | -X[:,:64]] is two contiguous half-copies.
Band data crosses HBM exactly once (except the lh pair, which is sent
pre-stacked 2x so its even/odd halves are partition-stacked for the PE).

Per slice, two PE stages of 4 matmuls each (256 moving rows, 1 cyc/row):
    stage 1: z^T[c',i] psum [128, 512]=[z1^T|z2^T]; stationaries Yl, A1
             (lh even;odd), A2 (hl;hh even), A3 (hl;hh odd) vs stacked
             row-parity statics
    stage 2: out[i-tile] = zs-half^T @ C*T_c (c'-row-permuted statics)

All bf16 with f32 PSUM accumulation; rel err ~6e-3 vs f32 ref (gate 2e-2).

Inputs ship in ONE packed dram tensor IN[p, slice, 5, 128] (Yl | U1 | V1 |
U2 | V2) = one dma_start per group (~650ns issue each), >=1.25KB
contiguous per-partition lines. Small starter/final groups shorten
pipeline fill/drain. Engine budget per slice: PE 8 matmuls, DVE 3 adds +
out-copy, ACT z-copy + 2 half-copies (A3), sync all DMA issues.
"""
import numpy as np
import ml_dtypes

import concourse.bacc as bacc
import concourse.tile as tile
from concourse import mybir

F32 = mybir.dt.float32
BF16 = mybir.dt.bfloat16
NPBF16 = ml_dtypes.bfloat16

PERM = np.concatenate([np.arange(0, 128, 2), np.arange(1, 128, 2)])

# ---------------- host-side static matrix construction ----------------

_H0A = np.array([0.0351638365171441, 0.0, -0.0883294244510729,
                 0.233890320607236, 0.760272369066126, 0.587518297723561,
                 0.0, -0.114301837144249, 0.0, 0.0], dtype=np.float64)
_H0B = _H0A[::-1].copy()
_ALT = (-1.0) ** np.arange(10)
_H1A = _H0B * _ALT
_H1B = _H1A[::-1].copy()
G0A, G0B, G1A, G1B = _H0B, _H0A, _H1B, _H1A


def _reflect(x, minx, maxx):
    x = np.asarray(x, dtype=np.float64)
    rng = maxx - minx
    rng2 = 2.0 * rng
    mod = np.fmod(x - minx, rng2)
    normed = np.where(mod < 0, mod + rng2, mod)
    return (np.where(normed >= rng, rng2 - normed, normed) + minx).astype(np.int64)


def _colifilt_matrix(ha, hb, r=128):
    """C (2r x r) with colifilt(X) = C @ X."""
    m = ha.shape[0]
    m2 = m // 2
    xe = _reflect(np.arange(-m2, r + m2), -0.5, r - 0.5)
    t = np.arange(2, r + m - 1, 2)
    if float(np.sum(ha * hb)) > 0:
        ta, tb = t, t - 1
    else:
        ta, tb = t - 1, t
    r2 = r // 2
    hao, hae = ha[0::2], ha[1::2]
    hbo, hbe = hb[0::2], hb[1::2]

    def vconv_mat(sel_idx, h):
        hf = h[::-1]
        M = np.zeros((r2, r), dtype=np.float64)
        for i in range(r2):
            for k in range(m2):
                M[i, sel_idx[i + k]] += hf[k]
        return M

    C = np.zeros((2 * r, r), dtype=np.float64)
    C[0::4] = vconv_mat(xe[tb], hao)
    C[1::4] = vconv_mat(xe[ta], hbo)
    C[2::4] = vconv_mat(xe[tb], hae)
    C[3::4] = vconv_mat(xe[ta], hbe)
    return C


def build_statics():
    """CT [128, 1536] bf16 = [C0T | C1s_EO | Cs_E2 | Cs_O2 | C0T_c | C1T_c]."""
    C0 = _colifilt_matrix(G0B, G0A)
    C1 = _colifilt_matrix(G1B, G1A)
    s = 1.0 / np.sqrt(2.0)
    C0T = C0.T
    C1T = C1.T
    C0sT = (s * C0).T
    C1sT = (s * C1).T
    C1s_EO = np.concatenate([C1sT[0::2], C1sT[1::2]], axis=0)
    Cs_E2 = np.concatenate([C0sT[0::2], C1sT[0::2]], axis=0)
    Cs_O2 = np.concatenate([C0sT[1::2], C1sT[1::2]], axis=0)
    CT = np.concatenate(
        [C0T, C1s_EO, Cs_E2, Cs_O2, C0T[PERM], C1T[PERM]], axis=1)
    return np.ascontiguousarray(CT.astype(np.float32).astype(NPBF16))


# ---------------- device kernel ----------------


def build_kernel(n_ch=64, G=8, n_cores=8):
    nc = bacc.Bacc("TRN2", target_bir_lowering=False, debug=False,
                   num_devices=n_cores)
    IN = nc.dram_tensor("IN", [128, 5, n_ch, 128], BF16, kind="ExternalInput").ap()
    CTD = nc.dram_tensor("CT", [128, 1536], BF16, kind="ExternalInput").ap()
    OUT = nc.dram_tensor("Y", [n_ch, 256, 256], BF16, kind="ExternalOutput").ap()

    # group schedule: tiny starter groups so the first matmuls fire early,
    # small final groups so the output-store drain tail is short
    groups = [(0, 2), (2, 2), (4, 4)]
    g0 = 8
    while g0 < n_ch - G:
        groups.append((g0, G))
        g0 += G
    groups += [(g0, 4), (g0 + 4, 2), (g0 + 6, 2)]
    assert sum(gl for _, gl in groups) == n_ch
    assert all(gl in (2, 4, 8) for _, gl in groups)

    with tile.TileContext(nc) as tc:
        with (
            tc.tile_pool(name="const", bufs=1) as const,
            tc.tile_pool(name="inp", bufs=3) as inp,
            tc.tile_pool(name="quad", bufs=3) as quad,
            tc.tile_pool(name="zt", bufs=4) as ztp,
            tc.tile_pool(name="yout", bufs=3) as yp,
            tc.tile_pool(name="psz", bufs=3, space="PSUM") as pp,
            tc.tile_pool(name="psy", bufs=3, space="PSUM") as ppy,
        ):
            ct = const.tile([128, 1536], BF16)
            nc.sync.dma_start(ct[:], CTD[:])
            C0T = ct[:, 0:256]
            C1s_EO = ct[:, 256:512]
            Cs_E2 = ct[:, 512:768]
            Cs_O2 = ct[:, 768:1024]
            C0Tc = ct[:, 1024:1280]
            C1Tc = ct[:, 1280:1536]

            def load_group(g0, Gl):
                tin = inp.tile([128, 5 * Gl * 128], BF16, tag=f"in{Gl}")
                nc.sync.dma_start(
                    tin.rearrange("p (k g c) -> p k g c", k=5, g=Gl),
                    IN[:, :, g0:g0 + Gl],
                )
                return tin, Gl

            def prep_group(state):
                tin, Gl = state
                S = Gl * 128
                # A-tiles: [A1 slices | A2 slices | A3 slices], contiguous
                AT = quad.tile([128, 3 * S], BF16, tag=f"at{Gl}")
                DT = quad.tile([128, S], BF16, tag=f"dt{Gl}")
                nc.vector.tensor_add(AT[:, 0:S], tin[:, S:2 * S], tin[:, 2 * S:3 * S])
                nc.vector.tensor_add(AT[:, S:2 * S], tin[:, 3 * S:4 * S],
                                     tin[:, 4 * S:5 * S])
                nc.vector.tensor_sub(DT[:], tin[:, 3 * S:4 * S], tin[:, 4 * S:5 * S])
                dv = DT.rearrange("p (g h c2) -> p g h c2", g=Gl, h=2)
                a3v = AT[:, 2 * S:3 * S].rearrange(
                    "p (g h c2) -> p g h c2", g=Gl, h=2)
                nc.scalar.copy(a3v[:, :, 0], dv[:, :, 1])
                nc.scalar.mul(a3v[:, :, 1], dv[:, :, 0], -1.0)
                return tin, AT, Gl

            def process_group(g0, state):
                tin, AT, Gl = state
                inv = tin.rearrange("p (k g c) -> p k g c", k=5, g=Gl)
                av = AT.rearrange("p (q g c) -> p q g c", q=3, g=Gl)
                half = min(Gl, 4)
                ybs = {}

                def emit_s1(ci):
                    zp = pp.tile([128, 512], F32, tag="zp")
                    nc.tensor.matmul(zp[:, 0:256], inv[:, 0, ci], C0T,
                                     start=True, stop=False, skip_group_check=True)
                    nc.tensor.matmul(zp[:, 0:256], av[:, 0, ci], C1s_EO,
                                     start=False, stop=False, skip_group_check=True)
                    nc.tensor.matmul(zp[:, 256:512], av[:, 1, ci], Cs_E2,
                                     start=False, stop=False, skip_group_check=True)
                    nc.tensor.matmul(zp[:, 256:512], av[:, 2, ci], Cs_O2,
                                     start=False, stop=True, skip_group_check=True)
                    zs = ztp.tile([128, 512], BF16, tag="zs")
                    nc.scalar.copy(zs[:], zp[:])
                    return zs

                def emit_s2(ci, zs):
                    op = ppy.tile([128, 512], F32, tag="op")
                    nc.tensor.matmul(op[:, 0:256], zs[:, 0:128], C0Tc,
                                     start=True, stop=False, skip_group_check=True)
                    nc.tensor.matmul(op[:, 0:256], zs[:, 256:384], C1Tc,
                                     start=False, stop=False, skip_group_check=True)
                    nc.tensor.matmul(op[:, 256:512], zs[:, 128:256], C0Tc,
                                     start=False, stop=False, skip_group_check=True)
                    nc.tensor.matmul(op[:, 256:512], zs[:, 384:512], C1Tc,
                                     start=False, stop=True, skip_group_check=True)
                    h0 = ci - ci % half
                    if h0 not in ybs:
                        YB = yp.tile([128, half * 512], BF16, tag=f"yb{half}")
                        ybs[h0] = YB
                    YB = ybs[h0]
                    nc.vector.tensor_copy(
                        YB[:, (ci - h0) * 512:(ci - h0 + 1) * 512], op[:])
                    if ci == h0 + half - 1:
                        nc.sync.dma_start(
                            OUT[g0 + h0:g0 + h0 + half].rearrange(
                                "g (it p) w -> p g it w", it=2),
                            YB.rearrange("p (g it w) -> p g it w",
                                         g=half, it=2),
                        )

                # stage-1 of slice ci+1 is emitted before stage-2 of slice
                # ci so the PE never waits on the psum->sbuf z-copy
                pending = None
                for ci in range(Gl):
                    zs = emit_s1(ci)
                    if pending is not None:
                        emit_s2(pending[0], pending[1])
                    pending = (ci, zs)
                emit_s2(pending[0], pending[1])

            # software pipeline: load + prep g+1 before computing g
            state = prep_group(load_group(*groups[0]))
            for idx, (g0, Gl) in enumerate(groups):
                if idx + 1 < len(groups):
                    nxt = prep_group(load_group(*groups[idx + 1]))
                    process_group(g0, state)
                    state = nxt
                else:
                    process_group(g0, state)

    nc.compile()
    return nc


# ---------------- host wrapper: shard, run on 8 cores, gather ----------------

_CACHED = {}


def _get_compiled():
    if "nc" not in _CACHED:
        _CACHED["nc"] = build_kernel(n_ch=64, G=8, n_cores=8)
        _CACHED["ct"] = build_statics()
    return _CACHED["nc"], _CACHED["ct"]


def _make_in_maps(Yl, Yhr, Yhi, CT):
    """Per-core input packing (pure layout: transpose/block/sign).
    IN[b] = [128(p), C, 5, 128]: k=0 Yl (cols blocked), k=1 U1, k=2 V1
    (lh pair, even/odd-stacked), k=3 U2=[Ra_hl;Ra_hh], k=4 V2=[Rb_hl;Rb_hh]."""
    B = Yl.shape[0]
    IN = np.empty((B, 128, 5, 64, 128), dtype=np.float32)
    IN[:, :, 0] = Yl[:, :, :, PERM].transpose(0, 2, 1, 3)

    def Rab(b):  # [B, 64(h), C, 128] = [wr | wi] blocked
        return np.concatenate(
            [Yhr[:, :, b], Yhi[:, :, b]], axis=-1).transpose(0, 2, 1, 3)

    def swapsig(X):
        return np.concatenate([X[..., 64:128], -X[..., 0:64]], axis=-1)

    Ra_lh, Rb_lh = Rab(0), Rab(5)
    IN[:, 0:64, 1] = Ra_lh
    IN[:, 64:128, 1] = swapsig(Ra_lh)
    IN[:, 0:64, 2] = Rb_lh
    IN[:, 64:128, 2] = -swapsig(Rb_lh)
    IN[:, 0:64, 3] = Rab(2)     # Ra_hl
    IN[:, 64:128, 3] = Rab(1)   # Ra_hh
    IN[:, 0:64, 4] = Rab(3)     # Rb_hl
    IN[:, 64:128, 4] = Rab(4)   # Rb_hh
    IN = IN.astype(NPBF16)
    return [{"IN": np.ascontiguousarray(IN[b]), "CT": CT} for b in range(B)]


def kernel(Yl, Yhr, Yhi):
    """Inverse DTCWT (qshift) level. Yl (8,64,128,128) f32,
    Yhr/Yhi (8,64,6,64,64) f32 -> (8,64,256,256) f32.
    Data-parallel over the batch dim: one batch element per NeuronCore."""
    from concourse.bass_utils import run_bass_kernel_spmd

    Yl = np.asarray(Yl, dtype=np.float32)
    Yhr = np.asarray(Yhr, dtype=np.float32)
    Yhi = np.asarray(Yhi, dtype=np.float32)
    B = Yl.shape[0]
    assert B == 8, f"expected batch 8, got {B}"

    nc, CT = _get_compiled()
    in_maps = _make_in_maps(Yl, Yhr, Yhi, CT)
    res = run_bass_kernel_spmd(nc, in_maps, core_ids=list(range(B)))
    out = np.stack([np.asarray(res.results[b]["Y"]) for b in range(B)])
    return out.astype(np.float32)
